# revision 8
# baseline (speedup 1.0000x reference)
"""AdaptedAttention (llama + adaption-prompt) on 8 TRN2 NeuronCores.

Sharding: tensor-parallel over heads (2 heads/core). Per core:
  - qT/kT/vT projections for its 2 heads from a bf16, pre-transposed X
    (all matmuls contract over d, so X lives on-chip as X.T [d part, s free])
  - RoPE applied in the [hd, s] layout (partition-half rotate)
  - attention computed as S^T = K @ Q^T tiles ([k part, q free]) so that
    softmax needs no transposes: exp on ACT (scale folded), row-sums via a
    ones-matmul on the TensorEngine, causal handled by skipping k-tiles
    above the diagonal + 0/1 masks on the 4 diagonal tiles
  - adapter path (L=10) in the same transposed layout; gate folded into the
    reciprocal broadcast (outer-product matmul)
  - ctx^T (2 heads, [256, s]) AllGathered across cores -> each core computes
    a 256-column slice of the output projection, returned transposed
Host side: slicing/transposing/casting of weights + final concat/transpose.
"""

import math
import numpy as np

import concourse.bass as bass
import concourse.bacc as bacc
import concourse.mybir as mybir
import concourse.tile as tile
from concourse.bass_utils import run_bass_kernel_spmd

BF16 = mybir.dt.bfloat16
F32 = mybir.dt.float32
NP_BF16 = mybir.dt.np(BF16)

TWO_PI = 2.0 * math.pi


class Cfg:
    def __init__(self, s=2048, d=2048, L=10, n_cores=8, n_heads=16, rope_base=10000.0):
        self.s, self.d, self.L = s, d, L
        self.n_cores = n_cores
        self.n_heads = n_heads
        self.rope_base = rope_base
        self.hd = 128                      # head dim (fixed)
        self.hpc = n_heads // n_cores      # heads per core
        self.dh = self.hpc * self.hd       # local head-dim cols per core
        self.nd = d // 128                 # contraction chunks
        self.QC = 512                      # q-chunk width
        self.ns = s // self.QC             # q-chunks
        self.nst = s // 128                # s tiles (k tiles)
        self.kpq = self.QC // 128          # k-tiles straddling one q-chunk diag
        assert self.hpc * n_cores == n_heads and d % 128 == 0 and s % self.QC == 0


def build(cfg: Cfg):
    """Build the per-core SPMD graph. Returns compiled nc."""
    c = cfg
    nc = bacc.Bacc(None, target_bir_lowering=False, num_devices=c.n_cores)

    # ---------------- external I/O (per-core shards) ----------------
    xt_d = nc.dram_tensor("xt", [c.d, c.s], BF16, kind="ExternalInput")
    wqt_d = nc.dram_tensor("wqt", [c.d, c.dh], BF16, kind="ExternalInput")
    wkt_d = nc.dram_tensor("wkt", [c.d, c.dh], BF16, kind="ExternalInput")
    wvt_d = nc.dram_tensor("wvt", [c.d, c.dh], BF16, kind="ExternalInput")
    wot_d = nc.dram_tensor("wot", [c.d, c.dh], BF16, kind="ExternalInput")
    apt_d = nc.dram_tensor("apt", [c.d, c.L], BF16, kind="ExternalInput")
    gatev_d = nc.dram_tensor("gatev", [1, 128], F32, kind="ExternalInput")
    posf_d = nc.dram_tensor("posf", [1, c.s], F32, kind="ExternalInput")
    out_d = nc.dram_tensor("out", [c.dh, c.s], F32, kind="ExternalOutput")

    # ---------------- compile-time constants ----------------
    inv_freq = 1.0 / (c.rope_base ** (np.arange(0, c.hd, 2, dtype=np.float64) / c.hd))
    invfreq_ext = np.concatenate([inv_freq, inv_freq]) / TWO_PI  # [128]
    invf_d = nc.inline_tensor(invfreq_ext[None, :].astype(np.float32), name="invf")

    # diagonal-tile causal masks: mask[k, j, q] = 1 if k <= q - 128*j
    kk = np.arange(128)[:, None, None]
    jj = np.arange(c.kpq)[None, :, None]
    qq = np.arange(c.QC)[None, None, :]
    masks_np = (kk <= qq - 128 * jj).astype(NP_BF16)  # [128, kpq, QC]
    masks_d = nc.inline_tensor(masks_np, name="masks")

    scale_s = 1.0 / math.sqrt(c.hd)        # main attention scale
    scale_a = 1.0 / math.sqrt(c.n_heads)   # adapter scale (faithful to ref)

    # AllGather bounce buffers (ctx^T local -> full)
    ag_in = nc.dram_tensor("ag_in", [c.dh, c.s], BF16)
    ag_space = "Shared" if c.n_cores > 4 else "Local"
    ag_out = nc.dram_tensor("ag_out", [c.d, c.s], BF16, addr_space=ag_space)
    rgroups = [list(range(c.n_cores))]

    with tile.TileContext(nc) as tc:
        with (
            tc.tile_pool(name="big", bufs=1) as bigp,
            tc.tile_pool(name="persist", bufs=1) as pp,
            tc.tile_pool(name="work", bufs=3) as wp,
            tc.tile_pool(name="psum", bufs=1, space="PSUM") as psp,
        ):
            # ---------- stage A: loads ----------
            xt = bigp.tile([128, c.nd, c.s], BF16, tag="big")
            nc.sync.dma_start(xt[:], xt_d.rearrange("(t p) s -> p t s", p=128))
            wqt = pp.tile([128, c.nd, c.dh], BF16, tag="wqt")
            wkt = pp.tile([128, c.nd, c.dh], BF16, tag="wkt")
            wvt = pp.tile([128, c.nd, c.dh], BF16, tag="wvt")
            wot = pp.tile([128, c.nd, c.dh], BF16, tag="wot")
            for t_sb, t_dr in ((wqt, wqt_d), (wkt, wkt_d), (wvt, wvt_d), (wot, wot_d)):
                nc.sync.dma_start(t_sb[:], t_dr.rearrange("(t p) m -> p t m", p=128))
            apt = pp.tile([128, c.nd, c.L], BF16, tag="apt")
            nc.sync.dma_start(apt[:], apt_d.rearrange("(t p) m -> p t m", p=128))
            gatev = pp.tile([1, 128], F32, tag="gatev")
            nc.sync.dma_start(gatev[:], gatev_d[:])
            posf = pp.tile([1, c.s], F32, tag="posf")
            nc.sync.dma_start(posf[:], posf_d[:])
            invf = pp.tile([1, 128], F32, tag="invf")
            nc.sync.dma_start(invf[:], invf_d[:])
            masks = pp.tile([128, c.kpq, c.QC], BF16, tag="masks")
            nc.sync.dma_start(masks[:], masks_d[:])
            ones_b = pp.tile([128, 1], BF16, tag="ones_b")
            nc.gpsimd.memset(ones_b[:], 1.0)
            ones_f = pp.tile([1, 128], F32, tag="ones_f")
            nc.gpsimd.memset(ones_f[:], 1.0)

            # ---------- stage B: rope tables cos/sin [128, s] f32 ----------
            cos_t = pp.tile([128, c.s], F32, tag="cos")
            sin_t = pp.tile([128, c.s], F32, tag="sin")
            MAGIC = 8388608.0  # 2^23: (u + 2^23) - 2^23 == round_to_nearest(u)
            ADD, SUB = mybir.AluOpType.add, mybir.AluOpType.subtract
            for qc in range(c.ns):
                sl = slice(qc * c.QC, (qc + 1) * c.QC)
                u_ps = psp.tile([128, c.QC], F32, tag="st")
                nc.tensor.matmul(u_ps[:], invf[:], posf[:, sl], start=True, stop=True)
                # sin(2*pi*u) with u - round(u) in [-1/2, 1/2]
                r1 = wp.tile([128, c.QC], F32, tag="tmp")
                nc.vector.tensor_scalar(r1[:], u_ps[:], MAGIC, -MAGIC, ADD, ADD)
                f1 = wp.tile([128, c.QC], F32, tag="tmp")
                nc.vector.tensor_tensor(f1[:], u_ps[:], r1[:], SUB)
                nc.scalar.activation(sin_t[:, sl], f1[:],
                                     mybir.ActivationFunctionType.Sin, scale=TWO_PI)
                # cos(2*pi*u) = sin(2*pi*(u+1/4)), same reduction
                a2 = wp.tile([128, c.QC], F32, tag="tmp")
                nc.vector.tensor_scalar(a2[:], u_ps[:], 0.25, MAGIC, ADD, ADD)
                r2 = wp.tile([128, c.QC], F32, tag="tmp")
                nc.vector.tensor_scalar(r2[:], a2[:], -MAGIC, None, ADD)
                f2 = wp.tile([128, c.QC], F32, tag="tmp")
                nc.vector.scalar_tensor_tensor(f2[:], u_ps[:], 0.25, r2[:], ADD, SUB)
                nc.scalar.activation(cos_t[:, sl], f2[:],
                                     mybir.ActivationFunctionType.Sin, scale=TWO_PI)

            # ---------- stage C: projections + rope ----------
            qrot = [pp.tile([128, c.s], BF16, tag=f"qrot{h}", name=f"qrot{h}")
                    for h in range(c.hpc)]
            krot = [pp.tile([128, c.s], BF16, tag=f"krot{h}", name=f"krot{h}")
                    for h in range(c.hpc)]
            v_sb = pp.tile([128, c.nst, c.dh], BF16, tag="v")

            def rope(dst, src_ps, sl):
                # dst[0:64]   = src[0:64]*cos[0:64] - src[64:]*sin[0:64]
                # dst[64:128] = src[64:]*cos[64:]   + src[0:64]*sin[64:]
                t1 = wp.tile([128, c.QC], F32, tag="tmp")
                t2 = wp.tile([128, c.QC], F32, tag="tmp")
                M = mybir.AluOpType.mult
                nc.vector.tensor_tensor(t1[0:64], src_ps[0:64], cos_t[0:64, sl], M)
                nc.vector.tensor_tensor(t2[0:64], src_ps[64:128], sin_t[0:64, sl], M)
                nc.vector.tensor_tensor(dst[0:64], t1[0:64], t2[0:64],
                                        mybir.AluOpType.subtract)
                nc.vector.tensor_tensor(t1[64:128], src_ps[64:128], cos_t[64:128, sl], M)
                nc.vector.tensor_tensor(t2[64:128], src_ps[0:64], sin_t[64:128, sl], M)
                nc.vector.tensor_tensor(dst[64:128], t1[64:128], t2[64:128],
                                        mybir.AluOpType.add)

            for qc in range(c.ns):
                sl = slice(qc * c.QC, (qc + 1) * c.QC)
                for h in range(c.hpc):
                    hsl = slice(h * 128, (h + 1) * 128)
                    q_ps = psp.tile([128, c.QC], F32, tag="qk")
                    k_ps = psp.tile([128, c.QC], F32, tag="qk")
                    for t in range(c.nd):
                        nc.tensor.matmul(q_ps[:], wqt[:, t, hsl], xt[:, t, sl],
                                         start=(t == 0), stop=(t == c.nd - 1))
                    for t in range(c.nd):
                        nc.tensor.matmul(k_ps[:], wkt[:, t, hsl], xt[:, t, sl],
                                         start=(t == 0), stop=(t == c.nd - 1))
                    rope(qrot[h][:, sl], q_ps, sl)
                    rope(krot[h][:, sl], k_ps, sl)
                # V natural [s, dh]
                for st in range(c.kpq):
                    gst = qc * c.kpq + st
                    ssl = slice(gst * 128, (gst + 1) * 128)
                    v_ps = psp.tile([128, c.dh], F32, tag="st")
                    for t in range(c.nd):
                        nc.tensor.matmul(v_ps[:], xt[:, t, ssl], wvt[:, t, :],
                                         start=(t == 0), stop=(t == c.nd - 1))
                    nc.scalar.copy(v_sb[:, gst, :], v_ps[:])

            # ---------- stage C2: adapter K/V ----------
            akt = pp.tile([128, c.hpc, c.L], BF16, tag="akt")
            av_sb = pp.tile([c.L, c.dh], BF16, tag="av")
            for h in range(c.hpc):
                hsl = slice(h * 128, (h + 1) * 128)
                a_ps = psp.tile([128, c.L], F32, tag="sum")
                for t in range(c.nd):
                    nc.tensor.matmul(a_ps[:], wkt[:, t, hsl], apt[:, t, :],
                                     start=(t == 0), stop=(t == c.nd - 1))
                nc.scalar.copy(akt[:, h, :], a_ps[:])
            av_ps = psp.tile([c.L, c.dh], F32, tag="sum")
            for t in range(c.nd):
                nc.tensor.matmul(av_ps[:], apt[:, t, :], wvt[:, t, :],
                                 start=(t == 0), stop=(t == c.nd - 1))
            nc.scalar.copy(av_sb[:], av_ps[:])

            # ---------- stage D: attention ----------
            EXP = mybir.ActivationFunctionType.Exp
            M = mybir.AluOpType.mult
            for h in range(c.hpc):
                for qc in range(c.ns):
                    sl = slice(qc * c.QC, (qc + 1) * c.QC)
                    nkt = qc * c.kpq + c.kpq  # causal: k-tiles 0..nkt-1
                    ctx_ps = psp.tile([128, c.QC], F32, tag="ctx")
                    sum_ps = psp.tile([1, c.QC], F32, tag="sum")
                    for kt in range(nkt):
                        ksl = slice(kt * 128, (kt + 1) * 128)
                        st_ps = psp.tile([128, c.QC], F32, tag="st")
                        nc.tensor.matmul(st_ps[:], krot[h][:, ksl], qrot[h][:, sl],
                                         start=True, stop=True)
                        est = wp.tile([128, c.QC], BF16, tag="est")
                        nc.scalar.activation(est[:], st_ps[:], EXP, scale=scale_s)
                        j = kt - qc * c.kpq
                        if j >= 0:
                            nc.vector.tensor_tensor(est[:], est[:], masks[:, j, :], M)
                        nc.tensor.matmul(ctx_ps[:], v_sb[:, kt, h * 128:(h + 1) * 128],
                                         est[:], start=(kt == 0), stop=(kt == nkt - 1))
                        nc.tensor.matmul(sum_ps[:], ones_b[:], est[:],
                                         start=(kt == 0), stop=(kt == nkt - 1))
                    # adapter
                    ast_ps = psp.tile([c.L, c.QC], F32, tag="sum")
                    nc.tensor.matmul(ast_ps[:], akt[:, h, :], qrot[h][:, sl],
                                     start=True, stop=True)
                    aest = wp.tile([c.L, c.QC], BF16, tag="aest")
                    nc.scalar.activation(aest[:], ast_ps[:], EXP, scale=scale_a)
                    actx_ps = psp.tile([128, c.QC], F32, tag="ctx")
                    nc.tensor.matmul(actx_ps[:], av_sb[:, h * 128:(h + 1) * 128],
                                     aest[:], start=True, stop=True)
                    asum_ps = psp.tile([1, c.QC], F32, tag="sum")
                    nc.tensor.matmul(asum_ps[:], ones_b[0:c.L, :], aest[:],
                                     start=True, stop=True)
                    # combine: ct = ctx/sum + gate*actx/asum
                    rc = wp.tile([1, c.QC], F32, tag="rc")
                    ra = wp.tile([1, c.QC], F32, tag="rc")
                    nc.vector.reciprocal(rc[:], sum_ps[:])
                    nc.vector.reciprocal(ra[:], asum_ps[:])
                    rcb_ps = psp.tile([128, c.QC], F32, tag="st")
                    nc.tensor.matmul(rcb_ps[:], ones_f[:], rc[:], start=True, stop=True)
                    rab_ps = psp.tile([128, c.QC], F32, tag="st")
                    nc.tensor.matmul(rab_ps[:], gatev[:], ra[:], start=True, stop=True)
                    rcb = wp.tile([128, c.QC], F32, tag="tmp")
                    rab = wp.tile([128, c.QC], F32, tag="tmp")
                    nc.scalar.copy(rcb[:], rcb_ps[:])
                    nc.scalar.copy(rab[:], rab_ps[:])
                    t1 = wp.tile([128, c.QC], F32, tag="tmp")
                    t2 = wp.tile([128, c.QC], F32, tag="tmp")
                    nc.vector.tensor_tensor(t1[:], ctx_ps[:], rcb[:], M)
                    nc.vector.tensor_tensor(t2[:], actx_ps[:], rab[:], M)
                    ct = wp.tile([128, c.QC], BF16, tag="ct")
                    nc.vector.tensor_tensor(ct[:], t1[:], t2[:], mybir.AluOpType.add)
                    nc.sync.dma_start(ag_in[h * 128:(h + 1) * 128, sl], ct[:])

            # ---------- stage E: AllGather ctx^T ----------
            nc.gpsimd.collective_compute(
                "AllGather", mybir.AluOpType.bypass,
                replica_groups=rgroups, ins=[ag_in[:]], outs=[ag_out[:]],
            )
            ctf = bigp.tile([128, c.nd, c.s], BF16, tag="big")
            nc.sync.dma_start(ctf[:], ag_out.rearrange("(t p) s -> p t s", p=128))

            # ---------- stage F: output projection (column slice, transposed) ----------
            for qc in range(c.ns):
                sl = slice(qc * c.QC, (qc + 1) * c.QC)
                for m in range(c.hpc):
                    msl = slice(m * 128, (m + 1) * 128)
                    o_ps = psp.tile([128, c.QC], F32, tag="ctx")
                    for t in range(c.nd):
                        nc.tensor.matmul(o_ps[:], wot[:, t, msl], ctf[:, t, sl],
                                         start=(t == 0), stop=(t == c.nd - 1))
                    o_sb = wp.tile([128, c.QC], F32, tag="tmp")
                    nc.scalar.copy(o_sb[:], o_ps[:])
                    nc.sync.dma_start(out_d[msl, sl], o_sb[:])

    nc.compile()
    return nc


def make_in_maps(cfg, hidden_states, Wq, Wk, Wv, Wo, adaption_prompt,
                 adaption_gate, position_ids):
    """Host-side sharding: slice/transpose/cast per core."""
    c = cfg
    x = np.asarray(hidden_states, np.float32)[0]          # [s, d]
    xt = np.ascontiguousarray(x.T).astype(NP_BF16)        # [d, s]
    ap = np.asarray(adaption_prompt, np.float32)[0]       # [L, d]
    apt = np.ascontiguousarray(ap.T).astype(NP_BF16)      # [d, L]
    gate = float(np.asarray(adaption_gate).reshape(-1)[0])
    gatev = np.full((1, 128), gate, np.float32)
    posf = np.asarray(position_ids).reshape(1, -1).astype(np.float32)
    in_maps = []
    for i in range(c.n_cores):
        rs = slice(i * c.dh, (i + 1) * c.dh)
        in_maps.append({
            "xt": xt,
            "wqt": np.ascontiguousarray(np.asarray(Wq, np.float32)[rs, :].T).astype(NP_BF16),
            "wkt": np.ascontiguousarray(np.asarray(Wk, np.float32)[rs, :].T).astype(NP_BF16),
            "wvt": np.ascontiguousarray(np.asarray(Wv, np.float32)[rs, :].T).astype(NP_BF16),
            "wot": np.ascontiguousarray(np.asarray(Wo, np.float32)[rs, :].T).astype(NP_BF16),
            "apt": apt,
            "gatev": gatev,
            "posf": posf,
        })
    return in_maps


def assemble_output(cfg, results):
    outs = [np.asarray(r["out"], np.float32) for r in results]
    big = np.concatenate(outs, axis=0)                    # [d, s]
    return np.ascontiguousarray(big.T)[None]              # [1, s, d]


_NC_CACHE = {}


def run(inputs, cfg=None, trace=False):
    cfg = cfg or Cfg()
    key = (cfg.s, cfg.d, cfg.L, cfg.n_cores, cfg.n_heads)
    if key not in _NC_CACHE:
        _NC_CACHE[key] = build(cfg)
    nc = _NC_CACHE[key]
    in_maps = make_in_maps(cfg, **inputs)
    res = run_bass_kernel_spmd(nc, in_maps, core_ids=list(range(cfg.n_cores)),
                               trace=trace)
    out = assemble_output(cfg, res.results)
    return out, res


def kernel(**inputs) -> np.ndarray:
    out, _ = run(inputs)
    return out.astype(np.float32)
